# revision 64
# baseline (speedup 1.0000x reference)
"""CaNetConv (GCN conv + gated multi-head linear) Trainium2 kernel.

Strategy (pull-mode graph SpMM, destinations sharded across 8 cores):
  hi[c,:] = sum_{e: col[e]=c} val[e] * x[row[e],:]      (GCN aggregation)
  out     = x + sum_k e[:,k] * (concat(hi,x) @ W[k])    (gated einsum)

Per core:
  - edges sorted by destination block, then by SOURCE row within each
    (block, half) bucket: the gather address stream is monotonic, which
    keeps the random 512B HBM reads DRAM-friendly (unsorted measures ~4x
    slower on HW once the einsum's concurrent HBM traffic drives access
    latency up)
  - x rows fetched f32 with gpsimd dma_gather (512B tokens = the SDMA
    line-rate threshold; bare bf16 256B tokens pay read-modify-write and
    measure ~3x slower), then cast to bf16 on the otherwise-idle Act
    engine so all matmuls run at 1 cycle/row instead of 4
  - DVE builds S[e,c] = (iota==colrel[e]) * val[e] in one tensor_scalar
    per 128-edge group (batched builds via 0-stride broadcast APs
    measured ~3x SLOWER on HW - keep the simple per-group op)
  - PE computes hiT[f,c] += msg[e,f]^T @ S[e,c] (segment sum as bf16
    matmul, accumulated in f32 PSUM per 128-dest block)
  - einsum as 4 bf16 matmuls per 128-node block vs preflattened W,
    gating sum via f32 scalar_tensor_tensor with per-partition e
    scalars, + f32 residual.  bf16 end-to-end rel err ~2e-3.

One NEFF shared by all 8 cores (SPMD): the static structure (gather run
lengths, groups per block) is padded to the max across cores.
"""

import sys

import numpy as np

for _p in ("/opt/trn_rl_repo", "/root/.axon_site/_ro/trn_rl_repo"):
    if _p not in sys.path:
        sys.path.append(_p)

import concourse.bass as bass  # noqa: E402
import concourse.tile as tile  # noqa: E402
from concourse import bacc, mybir  # noqa: E402

F32 = mybir.dt.float32
BF16 = mybir.dt.bfloat16
I16 = mybir.dt.int16

try:
    from ml_dtypes import bfloat16 as BF16NP
except ImportError:  # pragma: no cover
    import jax.numpy as _jnp
    BF16NP = _jnp.bfloat16

SUBRUN = 1024          # gather tokens per dma_gather; the SWDGE descriptor
                       # carveout (16KB/partition / 16B per desc) caps one
                       # gather at ~1024 descriptors — 1280 kills the device
SB_BLOCKS = 4          # dest blocks (128 dests) per gather superblock
PAD_COLREL = 200.0     # colrel sentinel that never matches iota 0..127


def _wrap16(a):
    """dma_gather index layout: [128, n/16], idx t at [t%16 (+16g), t//16]."""
    n = a.shape[0]
    assert n % 16 == 0
    w = a.reshape(n // 16, 16).T.astype(np.int16)  # [16, n/16]
    return np.tile(w, (8, 1))                      # replicated per Q7 core


def _prep(x, adj, e, weights, n_cores):
    """Host-side graph preprocessing. Returns (meta, in_maps)."""
    N, F = x.shape
    K = e.shape[1]
    E = adj.shape[1]
    row = np.asarray(adj[0], dtype=np.int64)
    col = np.asarray(adj[1], dtype=np.int64)

    NPC = N // n_cores                      # dests per core
    NB = (NPC + 127) // 128                 # 128-dest blocks per core
    NPCP = NB * 128
    NSB = (NB + SB_BLOCKS - 1) // SB_BLOCKS
    sb_nblocks = [min(SB_BLOCKS, NB - s * SB_BLOCKS) for s in range(NSB)]
    import os as _os
    n_half = 2 if (N > 32767 or _os.environ.get("KERNEL_FORCE_HALVES")) else 1
    HALF = (N + 1) // 2 if n_half == 2 else N

    # GCN normalization (destination degree), f32 like the reference
    deg = np.bincount(col, minlength=N).astype(np.float32)
    with np.errstate(divide="ignore"):
        r = 1.0 / np.sqrt(deg)
    r[~np.isfinite(r)] = 0.0
    val_e = (r[col] * r[row]).astype(np.float32)

    # per-core edge lists sorted by local dest
    cores = []
    counts = np.zeros((n_cores, NB, n_half), dtype=np.int64)
    for c in range(n_cores):
        m = (col >= c * NPC) & (col < (c + 1) * NPC)
        # rotate sources into core-local coordinates: core c's x copy is
        # rolled so its own dest rows start at 0, letting the residual read
        # straight from x_full at SPMD-uniform offsets (no x_res input)
        rc = (row[m] - c * NPC) % N
        cc = col[m] - c * NPC
        vc = val_e[m]
        o = np.argsort(cc, kind="stable")
        rc, cc, vc = rc[o], cc[o], vc[o]
        blk = cc >> 7
        half = (rc >= HALF).astype(np.int64)
        # bucket edges per (block, half), preserving order
        key = blk * n_half + half
        ob = np.argsort(key, kind="stable")
        rc, cc, vc, blk, half = rc[ob], cc[ob], vc[ob], blk[ob], half[ob]
        np.add.at(counts[c], (blk, half), 1)
        cores.append((rc, cc, vc, blk, half))

    # static structure: groups per (block, half) = max over cores
    G = np.maximum.reduce([np.ceil(counts[c] / 128.0).astype(np.int64)
                           for c in range(n_cores)])
    for b in range(NB):
        if G[b].sum() == 0:
            G[b, 0] = 1  # guarantee every block's psum region is written

    # Token stream AND matmul emission are block-major (for b: for h:
    # groups): each (block, half) span is one gather run (single source
    # half), each block's PSUM accumulation group is contiguous, and msg
    # tiles are consumed in stream order (bounded pool pressure).
    sched = []          # per sb: [(token_gidx, b_loc, start, stop), ...]
    sb_runs = []        # per sb: [(h, ntokens), ...] gather runs in order
    for s in range(NSB):
        blocks = list(range(s * SB_BLOCKS, s * SB_BLOCKS + sb_nblocks[s]))
        groups = []
        runs = []
        for bi, b in enumerate(blocks):
            # alternate the half order per block so adjacent blocks' runs
            # share a source half and merge into one gather run below
            order = ((0, 1) if (bi % 2 == 0 or n_half == 1) else (1, 0))[:n_half]
            metas = []
            for h in order:
                if G[b, h] > 0:
                    if runs and runs[-1][0] == h:
                        runs[-1][1] += int(G[b, h]) * 128
                    else:
                        runs.append([h, int(G[b, h]) * 128])
                for _ in range(G[b, h]):
                    metas.append([len(groups) + len(metas),
                                  b - s * SB_BLOCKS, False, False])
            metas[0][2] = True
            metas[-1][3] = True
            groups.extend(metas)
        sched.append(groups)
        sb_runs.append(runs)

    G_total = sum(len(g) for g in sched)

    # per-core token arrays in schedule order
    in_maps = []
    xpad = np.zeros((N + 128, F), dtype=np.float32)
    xpad[:N] = x
    epad = np.zeros((N + 128, K), dtype=np.float32)
    epad[:N] = e
    WF = np.ascontiguousarray(
        weights.astype(np.float32).transpose(1, 0, 2).reshape(2 * F, K * F))
    W_dram = np.concatenate([WF[:F], WF[F:]], axis=1)  # [128, 2*K*F]

    for c in range(n_cores):
        rc, cc, vc, blk, half = cores[c]
        idx_parts, colrel_parts, val_parts = [], [], []
        for s in range(NSB):
            blocks = range(s * SB_BLOCKS, s * SB_BLOCKS + sb_nblocks[s])
            for bi, b in enumerate(blocks):
                for h in ((0, 1) if (bi % 2 == 0 or n_half == 1) else (1, 0))[:n_half]:
                    m = (blk == b) & (half == h)
                    ridx = rc[m] - h * HALF
                    crel = (cc[m] - b * 128).astype(np.float32)
                    v = vc[m]
                    # sort by source row: monotonic gather addresses keep
                    # the DMA access pattern DRAM-friendly (random 512B
                    # reads over the full x cost ~4x on HW)
                    o_src = np.argsort(ridx, kind="stable")
                    ridx, crel, v = ridx[o_src], crel[o_src], v[o_src]
                    n = m.sum()
                    npad = G[b, h] * 128 - n
                    assert npad >= 0
                    idx_parts.append(np.concatenate(
                        [ridx, np.zeros(npad, np.int64)]).astype(np.int16))
                    colrel_parts.append(np.concatenate(
                        [crel, np.full(npad, PAD_COLREL, np.float32)]))
                    val_parts.append(np.concatenate(
                        [v, np.zeros(npad, np.float32)]))
        idx_cat = np.concatenate(idx_parts)
        colrel_cat = np.concatenate(colrel_parts)
        val_cat = np.concatenate(val_parts)
        if _os.environ.get("KERNEL_CLAMP_IDX"):
            idx_cat = (idx_cat % int(_os.environ["KERNEL_CLAMP_IDX"])).astype(np.int16)
        assert idx_cat.shape[0] == G_total * 128

        # wrapped idx stream, sliced per (sb, block, half) gather sub-run
        idx_w = []
        off = 0
        for s in range(NSB):
            for _h, ntok in sb_runs[s]:
                rem = ntok
                while rem > 0:
                    take = min(SUBRUN, rem)
                    idx_w.append(_wrap16(idx_cat[off:off + take]))
                    off += take
                    rem -= take
        idx_dram = (np.concatenate(idx_w, axis=1) if idx_w
                    else np.zeros((128, 16), np.int16))
        idx_cols = idx_dram.shape[1]

        colrel_dram = np.ascontiguousarray(
            colrel_cat.reshape(G_total, 128).T)
        val_dram = np.ascontiguousarray(val_cat.reshape(G_total, 128).T)

        xT = np.ascontiguousarray(xpad[c * NPC:c * NPC + NPCP].T)
        xroll = np.concatenate([xpad[c * NPC:N], xpad[:c * NPC], xpad[N:]])
        e_gate = np.ascontiguousarray(
            epad[c * NPC:c * NPC + NPCP].reshape(NB, 128, K)
            .transpose(1, 0, 2).reshape(128, NB * K))

        # every [128, X] constant packed into ONE int16 tensor: each bound
        # NEFF tensor costs ~28 us of dispatch overhead per execution, so
        # 7 inputs -> 1 is worth ~170 us/exec.  f32 regions first (4-byte
        # alignment), then bf16/i16; device side reads bitcast views.
        pack = np.concatenate([
            colrel_dram.view(np.int16),
            val_dram.view(np.int16),
            e_gate.view(np.int16),
            np.ascontiguousarray(xT.astype(BF16NP)).view(np.int16),
            np.ascontiguousarray(W_dram.astype(BF16NP)).view(np.int16),
            np.ascontiguousarray(
                np.tile(np.arange(128, dtype=np.float32),
                        (128, 1)).astype(BF16NP)).view(np.int16),
            np.ascontiguousarray(idx_dram),
        ], axis=1)

        in_maps.append({
            "x_full": np.ascontiguousarray(xroll),
            "pack": pack,
        })

    meta = dict(N=N, F=F, K=K, E=E, NPC=NPC, NB=NB, NPCP=NPCP, NSB=NSB,
                sb_nblocks=sb_nblocks, n_half=n_half, HALF=HALF,
                sched=sched, sb_runs=sb_runs, G_total=G_total,
                idx_cols=idx_cols,
                n_cores=n_cores)
    return meta, in_maps


def _build(meta):
    """Trace the Bass/Tile kernel for the static structure in meta."""
    from contextlib import ExitStack

    N, F, K = meta["N"], meta["F"], meta["K"]
    NB, NPCP, NSB = meta["NB"], meta["NPCP"], meta["NSB"]
    sb_nblocks, n_half, HALF = meta["sb_nblocks"], meta["n_half"], meta["HALF"]
    sched, sb_runs = meta["sched"], meta["sb_runs"]

    nc = bacc.Bacc("TRN2", target_bir_lowering=False, debug=False,
                   num_devices=meta["n_cores"], num_swdge_queues=4)

    x_full = nc.dram_tensor("x_full", [N + 128, F], F32, kind="ExternalInput")
    KF = K * F
    G = meta["G_total"]
    # packed constants, int16 columns (see _prep): colrel f32 | val f32 |
    # e_gate f32 | xT bf16 | W bf16 | iota bf16 | idx i16
    o_cr = 0
    o_val = o_cr + 2 * G
    o_eg = o_val + 2 * G
    o_xT = o_eg + 2 * NB * K
    o_w = o_xT + NPCP
    o_iota = o_w + 2 * KF
    o_idx = o_iota + 128
    PACK_C = o_idx + meta["idx_cols"]
    pack_d = nc.dram_tensor("pack", [128, PACK_C], I16, kind="ExternalInput")
    out_d = nc.dram_tensor("out", [NPCP, F], F32, kind="ExternalOutput")

    assert KF == 1024 and F == 128, "einsum slicing hardcoded for K=8, F=128"

    with tile.TileContext(nc) as tc, ExitStack() as ctx:
        const = ctx.enter_context(tc.tile_pool(name="const", bufs=1))
        msgp = ctx.enter_context(tc.tile_pool(name="msg", bufs=14))
        msgbp = ctx.enter_context(tc.tile_pool(name="msgb", bufs=14))
        sp = ctx.enter_context(tc.tile_pool(name="sp", bufs=16))
        hiTp = ctx.enter_context(tc.tile_pool(name="hiT", bufs=NSB))
        accp = ctx.enter_context(tc.tile_pool(name="acc", bufs=6))
        psag = ctx.enter_context(tc.tile_pool(name="psag", bufs=3,
                                              space="PSUM"))
        psmm = ctx.enter_context(tc.tile_pool(name="psmm", bufs=4,
                                              space="PSUM"))

        # idx cols consumed by each superblock's gathers (for chunked loads)
        sb_idx_cols = []
        for s in range(NSB):
            cols = 0
            for _h, ntok in sb_runs[s]:
                rem = ntok
                while rem > 0:
                    take = min(SUBRUN, rem)
                    cols += take // 16
                    rem -= take
            sb_idx_cols.append(cols)

        # one persistent packed-constant tile; the idx region is loaded in
        # per-superblock chunks (first chunk first) so early gathers don't
        # wait on the full idx stream or the other constant loads.
        pack_t = const.tile([128, PACK_C], I16, tag="pack")
        off = 0
        for s in range(NSB):
            if sb_idx_cols[s]:
                nc.sync.dma_start(
                    pack_t[:, o_idx + off:o_idx + off + sb_idx_cols[s]],
                    pack_d.ap()[:, o_idx + off:o_idx + off + sb_idx_cols[s]])
                off += sb_idx_cols[s]
            if s == 0:
                nc.sync.dma_start(pack_t[:, 0:o_idx],
                                  pack_d.ap()[:, 0:o_idx])

        iota_v = pack_t[:, o_iota:o_iota + 128].bitcast(BF16)

        def cr_sc(g):
            return pack_t[:, o_cr + 2 * g:o_cr + 2 * g + 2].bitcast(F32)

        def val_sc(g):
            return pack_t[:, o_val + 2 * g:o_val + 2 * g + 2].bitcast(F32)

        def eg_sc(col):
            return pack_t[:, o_eg + 2 * col:o_eg + 2 * col + 2].bitcast(F32)

        def xT_view(a, b):
            return pack_t[:, o_xT + a:o_xT + b].bitcast(BF16)

        def w_view(a, b):
            return pack_t[:, o_w + a:o_w + b].bitcast(BF16)

        x_half = [x_full.ap()[h * HALF:N + 128, :] for h in range(n_half)]

        import os as _os
        _gather_only = _os.environ.get("KERNEL_GATHER_ONLY")
        _plain_dma = _os.environ.get("KERNEL_PLAIN_DMA")

        g_base = 0
        idx_off = 0
        n_gathers = 0
        for s in range(NSB):
            nb = sb_nblocks[s]
            groups = sched[s]
            ps_hi = psag.tile([128, nb * 128], F32, tag="psag")

            # gather msg tiles: one run per (block, half), in SUBRUN chunks
            tok_map = []            # token-order group idx -> (tile, slot)
            for h, ntok in sb_runs[s]:
                rem = ntok
                while rem > 0:
                    take = min(SUBRUN, rem)
                    mt = msgp.tile([128, take // 128, F], F32, tag="msg")
                    assert not _plain_dma, "plain-DMA probe retired"
                    nc.gpsimd.dma_gather(
                        mt[:], x_half[h],
                        pack_t[:, o_idx + idx_off:o_idx + idx_off + take // 16],
                        take, take, F, queue_num=n_gathers % 4)
                    n_gathers += 1
                    # cast to bf16 on the (otherwise idle) Act engine so the
                    # aggregation matmuls run at 1 cycle/row instead of 4
                    mb = msgbp.tile([128, take // 128, F], BF16, tag="msgb")
                    nc.scalar.copy(mb[:], mt[:])
                    for j in range(take // 128):
                        tok_map.append((mb, j))
                    idx_off += take // 16
                    rem -= take

            if _gather_only:
                for b_loc in range(nb):
                    b = s * SB_BLOCKS + b_loc
                    acc = accp.tile([128, F], F32, tag="acc")
                    nc.vector.tensor_copy(acc[:], iota_v)
                    nc.sync.dma_start(out_d.ap()[b * 128:(b + 1) * 128, :],
                                      acc[:])
                g_base += len(groups)
                continue

            # S build + aggregation matmuls, block-major emission order
            for tg, b_loc, start, stop in groups:
                g = g_base + tg
                s_t = sp.tile([128, 128], BF16, tag="s")
                nc.vector.tensor_scalar(
                    s_t[:], iota_v, cr_sc(g), val_sc(g),
                    mybir.AluOpType.is_equal, mybir.AluOpType.mult)
                mt, j = tok_map[tg]
                nc.tensor.matmul(
                    ps_hi[:, b_loc * 128:(b_loc + 1) * 128],
                    mt[:, j:j + 1, :], s_t[:], start=start, stop=stop)
            g_base += len(groups)

            hiT_t = hiTp.tile([128, nb * 128], BF16, tag="hiT")
            nc.vector.tensor_copy(hiT_t[:], ps_hi[:])

            import os as _os
            if _os.environ.get("KERNEL_SKIP_EINSUM"):
                for b_loc in range(nb):
                    b = s * SB_BLOCKS + b_loc
                    acc = accp.tile([128, F], F32, tag="acc")
                    nc.vector.tensor_copy(acc[:],
                                          hiT_t[:, b_loc * 128:(b_loc + 1) * 128])
                    nc.sync.dma_start(out_d.ap()[b * 128:(b + 1) * 128, :],
                                      acc[:])
                continue

            # einsum + gating + residual per block
            for b_loc in range(nb):
                b = s * SB_BLOCKS + b_loc
                hiT_b = hiT_t[:, b_loc * 128:(b_loc + 1) * 128]
                xT_b = xT_view(b * 128, (b + 1) * 128)
                pa = psmm.tile([128, 512], F32, tag="pmm")
                pb = psmm.tile([128, 512], F32, tag="pmm")
                nc.tensor.matmul(pa[:], hiT_b, w_view(0, 512),
                                 start=True, stop=False)
                nc.tensor.matmul(pb[:], hiT_b, w_view(512, 1024),
                                 start=True, stop=False)
                nc.tensor.matmul(pa[:], xT_b, w_view(1024, 1536),
                                 start=False, stop=True)
                nc.tensor.matmul(pb[:], xT_b, w_view(1536, 2048),
                                 start=False, stop=True)
                acc = accp.tile([128, F], F32, tag="acc")
                nc.sync.dma_start(acc[:], x_full.ap()[b * 128:(b + 1) * 128, :])
                for k in range(K):
                    src = pa if k < 4 else pb
                    kk = k % 4
                    nc.vector.scalar_tensor_tensor(
                        acc[:], src[:, kk * 128:(kk + 1) * 128],
                        eg_sc(b * K + k), acc[:],
                        mybir.AluOpType.mult, mybir.AluOpType.add)
                nc.sync.dma_start(out_d.ap()[b * 128:(b + 1) * 128, :], acc[:])

    nc.compile()
    return nc


def _bench(nc, in_maps, n_cores, k_lo=2, k_hi=512, reps=3):
    """Amortized per-execution wall time of the compiled NEFF on the axon
    cores (inputs staged on device, pipelined async dispatches). Upper
    bound: includes axon per-dispatch overhead (~2.5-3 ms amortized).
    Returns (per_exec_ns, results_list)."""
    import time

    import jax
    from jax.sharding import Mesh, PartitionSpec
    from jax.experimental.shard_map import shard_map

    from concourse import bass2jax, mybir as _mb
    from concourse.bass2jax import _bass_exec_p, partition_id_tensor

    bass2jax.install_neuronx_cc_hook()

    partition_name = (nc.partition_id_tensor.name
                      if nc.partition_id_tensor else None)
    in_names, out_names, out_avals, zero_outs = [], [], [], []
    for alloc in nc.m.functions[0].allocations:
        if not isinstance(alloc, _mb.MemoryLocationSet):
            continue
        name = alloc.memorylocations[0].name
        if alloc.kind == "ExternalInput":
            if name != partition_name:
                in_names.append(name)
        elif alloc.kind == "ExternalOutput":
            shape = tuple(alloc.tensor_shape)
            dtype = _mb.dt.np(alloc.dtype)
            out_names.append(name)
            out_avals.append(jax.core.ShapedArray(shape, dtype))
            zero_outs.append(np.zeros(shape, dtype))
    n_params = len(in_names)
    all_in_names = in_names + out_names
    if partition_name is not None:
        all_in_names = all_in_names + [partition_name]
    def _body(*args):
        operands = list(args)
        if partition_name is not None:
            operands.append(partition_id_tensor())
        return tuple(_bass_exec_p.bind(
            *operands, out_avals=tuple(out_avals),
            in_names=tuple(all_in_names), out_names=tuple(out_names),
            lowering_input_output_aliases=(), sim_require_finite=True,
            sim_require_nnan=True, nc=nc))

    devices = jax.devices()[:n_cores]
    mesh = Mesh(np.asarray(devices), ("core",))
    nin = n_params + len(out_names)
    sh = jax.sharding.NamedSharding(mesh, PartitionSpec("core"))
    concat_in = [jax.device_put(
        np.concatenate([np.asarray(in_maps[c][k]) for c in range(n_cores)], 0),
        sh) for k in in_names]
    concat_zeros = [jax.device_put(
        np.zeros((n_cores * z.shape[0], *z.shape[1:]), z.dtype), sh)
        for z in zero_outs]
    fn = jax.jit(shard_map(_body, mesh=mesh,
                           in_specs=(PartitionSpec("core"),) * nin,
                           out_specs=(PartitionSpec("core"),) * len(out_names),
                           check_rep=False), keep_unused=True)
    out = fn(*concat_in, *concat_zeros)   # warmup (compile+load)
    jax.block_until_ready(out)
    iters = k_hi
    best = float("inf")
    for _ in range(reps):
        t0 = time.perf_counter()
        for _ in range(iters):
            out = fn(*concat_in, *concat_zeros)
        jax.block_until_ready(out)
        best = min(best, (time.perf_counter() - t0) / iters)
    results = [{name: np.asarray(out[i]).reshape(n_cores, *out_avals[i].shape)[c]
                for i, name in enumerate(out_names)} for c in range(n_cores)]
    return best * 1e9, results


def _run(x, adj, e, weights, n_cores=8, sim=False, trace=False):
    meta, in_maps = _prep(x, adj, e, weights, n_cores)
    nc = _build(meta)
    N, F, NPC, NPCP = meta["N"], meta["F"], meta["NPC"], meta["NPCP"]

    if sim:
        from concourse.bass_interp import CoreSim
        outs = []
        for c in range(n_cores):
            simr = CoreSim(nc)
            for k, v in in_maps[c].items():
                simr.tensor(k)[:] = v
            simr.simulate(check_with_hw=False)
            outs.append(np.array(simr.tensor("out")))
        res = None
    elif trace:
        per_iter_ns, results = _bench(nc, in_maps, n_cores)
        outs = [r["out"] for r in results]
        res = per_iter_ns
    else:
        from concourse.bass_utils import run_bass_kernel_spmd
        res = run_bass_kernel_spmd(nc, in_maps, core_ids=list(range(n_cores)),
                                   trace=trace)
        outs = [r["out"] for r in res.results]

    out = np.concatenate([o[:NPC] for o in outs], axis=0)
    assert out.shape == (N, F)
    return out.astype(np.float32), res


def kernel(x, adj, e, weights):
    x = np.asarray(x, dtype=np.float32)
    adj = np.asarray(adj)
    e = np.asarray(e, dtype=np.float32)
    weights = np.asarray(weights, dtype=np.float32)
    out, _ = _run(x, adj, e, weights, n_cores=8, sim=False)
    return out



# revision 69
# speedup vs baseline: 1.0429x; 1.0429x over previous
"""CaNetConv (GCN conv + gated multi-head linear) Trainium2 kernel.

Strategy (pull-mode graph SpMM, destinations sharded across 8 cores):
  hi[c,:] = sum_{e: col[e]=c} val[e] * x[row[e],:]      (GCN aggregation)
  out     = x + sum_k e[:,k] * (concat(hi,x) @ W[k])    (gated einsum)

Per core:
  - edges sorted by destination block, then by SOURCE row within each
    (block, half) bucket: the gather address stream is monotonic, which
    keeps the random 512B HBM reads DRAM-friendly (unsorted measures ~4x
    slower on HW once the einsum's concurrent HBM traffic drives access
    latency up)
  - x rows fetched f32 with gpsimd dma_gather (512B tokens = the SDMA
    line-rate threshold; bare bf16 256B tokens pay read-modify-write and
    measure ~3x slower), then cast to bf16 on the otherwise-idle Act
    engine so all matmuls run at 1 cycle/row instead of 4
  - DVE builds S[e,c] = (iota==colrel[e]) * val[e] in one tensor_scalar
    per 128-edge group (batched builds via 0-stride broadcast APs
    measured ~3x SLOWER on HW - keep the simple per-group op)
  - PE computes hiT[f,c] += msg[e,f]^T @ S[e,c] (segment sum as bf16
    matmul, accumulated in f32 PSUM per 128-dest block)
  - einsum as 4 bf16 matmuls per 128-node block vs preflattened W,
    gating sum via f32 scalar_tensor_tensor with per-partition e
    scalars, + f32 residual.  bf16 end-to-end rel err ~2e-3.

One NEFF shared by all 8 cores (SPMD): the static structure (gather run
lengths, groups per block) is padded to the max across cores.

Dispatch-overhead engineering (each bound NEFF tensor costs ~28 us of
per-execution dispatch overhead through the axon RPC stack):
  - all [128, X] constants (colrel/val/e_gate/xT/W/iota/idx) live in ONE
    packed int16 input read through bitcast views on device
  - each core's x copy is rotated so its own dest rows start at row 0 -
    the residual loads straight from x_full (no separate x_res input)
  - net: 3 inputs + 1 output bound per execution (was 11 + 2)
"""

import sys

import numpy as np

for _p in ("/opt/trn_rl_repo", "/root/.axon_site/_ro/trn_rl_repo"):
    if _p not in sys.path:
        sys.path.append(_p)

import concourse.bass as bass  # noqa: E402
import concourse.tile as tile  # noqa: E402
from concourse import bacc, mybir  # noqa: E402

F32 = mybir.dt.float32
BF16 = mybir.dt.bfloat16
I16 = mybir.dt.int16

try:
    from ml_dtypes import bfloat16 as BF16NP
except ImportError:  # pragma: no cover
    import jax.numpy as _jnp
    BF16NP = _jnp.bfloat16

SUBRUN = 1024          # gather tokens per dma_gather; the SWDGE descriptor
                       # carveout (16KB/partition / 16B per desc) caps one
                       # gather at ~1024 descriptors — 1280 kills the device
SB_BLOCKS = 4          # dest blocks (128 dests) per gather superblock
PAD_COLREL = 200.0     # colrel sentinel that never matches iota 0..127


def _wrap16(a):
    """dma_gather index layout: [128, n/16], idx t at [t%16 (+16g), t//16]."""
    n = a.shape[0]
    assert n % 16 == 0
    w = a.reshape(n // 16, 16).T.astype(np.int16)  # [16, n/16]
    return np.tile(w, (8, 1))                      # replicated per Q7 core


def _prep(x, adj, e, weights, n_cores):
    """Host-side graph preprocessing. Returns (meta, in_maps)."""
    N, F = x.shape
    K = e.shape[1]
    E = adj.shape[1]
    row = np.asarray(adj[0], dtype=np.int64)
    col = np.asarray(adj[1], dtype=np.int64)

    NPC = N // n_cores                      # dests per core
    NB = (NPC + 127) // 128                 # 128-dest blocks per core
    NPCP = NB * 128
    NSB = (NB + SB_BLOCKS - 1) // SB_BLOCKS
    sb_nblocks = [min(SB_BLOCKS, NB - s * SB_BLOCKS) for s in range(NSB)]
    import os as _os
    n_half = 2 if (N > 32767 or _os.environ.get("KERNEL_FORCE_HALVES")) else 1
    HALF = (N + 1) // 2 if n_half == 2 else N

    # GCN normalization (destination degree), f32 like the reference
    deg = np.bincount(col, minlength=N).astype(np.float32)
    with np.errstate(divide="ignore"):
        r = 1.0 / np.sqrt(deg)
    r[~np.isfinite(r)] = 0.0
    val_e = (r[col] * r[row]).astype(np.float32)

    # per-core edge lists sorted by local dest
    cores = []
    counts = np.zeros((n_cores, NB, n_half), dtype=np.int64)
    for c in range(n_cores):
        m = (col >= c * NPC) & (col < (c + 1) * NPC)
        # rotate sources into core-local coordinates: core c's x copy is
        # rolled so its own dest rows start at 0, letting the residual read
        # straight from x_full at SPMD-uniform offsets (no x_res input)
        rc = (row[m] - c * NPC) % N
        cc = col[m] - c * NPC
        vc = val_e[m]
        o = np.argsort(cc, kind="stable")
        rc, cc, vc = rc[o], cc[o], vc[o]
        blk = cc >> 7
        half = (rc >= HALF).astype(np.int64)
        # bucket edges per (block, half), preserving order
        key = blk * n_half + half
        ob = np.argsort(key, kind="stable")
        rc, cc, vc, blk, half = rc[ob], cc[ob], vc[ob], blk[ob], half[ob]
        np.add.at(counts[c], (blk, half), 1)
        cores.append((rc, cc, vc, blk, half))

    # static structure: groups per (block, half) = max over cores
    G = np.maximum.reduce([np.ceil(counts[c] / 128.0).astype(np.int64)
                           for c in range(n_cores)])
    for b in range(NB):
        if G[b].sum() == 0:
            G[b, 0] = 1  # guarantee every block's psum region is written

    # Token stream AND matmul emission are block-major (for b: for h:
    # groups): each (block, half) span is one gather run (single source
    # half), each block's PSUM accumulation group is contiguous, and msg
    # tiles are consumed in stream order (bounded pool pressure).
    sched = []          # per sb: [(token_gidx, b_loc, start, stop), ...]
    sb_runs = []        # per sb: [(h, ntokens), ...] gather runs in order
    for s in range(NSB):
        blocks = list(range(s * SB_BLOCKS, s * SB_BLOCKS + sb_nblocks[s]))
        groups = []
        runs = []
        for bi, b in enumerate(blocks):
            # alternate the half order per block so adjacent blocks' runs
            # share a source half and merge into one gather run below
            order = ((0, 1) if (bi % 2 == 0 or n_half == 1) else (1, 0))[:n_half]
            metas = []
            for h in order:
                if G[b, h] > 0:
                    if runs and runs[-1][0] == h:
                        runs[-1][1] += int(G[b, h]) * 128
                    else:
                        runs.append([h, int(G[b, h]) * 128])
                for _ in range(G[b, h]):
                    metas.append([len(groups) + len(metas),
                                  b - s * SB_BLOCKS, False, False])
            metas[0][2] = True
            metas[-1][3] = True
            groups.extend(metas)
        sched.append(groups)
        sb_runs.append(runs)

    G_total = sum(len(g) for g in sched)

    # per-core token arrays in schedule order
    in_maps = []
    xpad = np.zeros((N + 128, F), dtype=np.float32)
    xpad[:N] = x
    epad = np.zeros((N + 128, K), dtype=np.float32)
    epad[:N] = e
    WF = np.ascontiguousarray(
        weights.astype(np.float32).transpose(1, 0, 2).reshape(2 * F, K * F))
    W_dram = np.concatenate([WF[:F], WF[F:]], axis=1)  # [128, 2*K*F]

    for c in range(n_cores):
        rc, cc, vc, blk, half = cores[c]
        idx_parts, colrel_parts, val_parts = [], [], []
        for s in range(NSB):
            blocks = range(s * SB_BLOCKS, s * SB_BLOCKS + sb_nblocks[s])
            for bi, b in enumerate(blocks):
                for h in ((0, 1) if (bi % 2 == 0 or n_half == 1) else (1, 0))[:n_half]:
                    m = (blk == b) & (half == h)
                    ridx = rc[m] - h * HALF
                    crel = (cc[m] - b * 128).astype(np.float32)
                    v = vc[m]
                    # sort by source row: monotonic gather addresses keep
                    # the DMA access pattern DRAM-friendly (random 512B
                    # reads over the full x cost ~4x on HW)
                    o_src = np.argsort(ridx, kind="stable")
                    ridx, crel, v = ridx[o_src], crel[o_src], v[o_src]
                    n = m.sum()
                    npad = G[b, h] * 128 - n
                    assert npad >= 0
                    idx_parts.append(np.concatenate(
                        [ridx, np.zeros(npad, np.int64)]).astype(np.int16))
                    colrel_parts.append(np.concatenate(
                        [crel, np.full(npad, PAD_COLREL, np.float32)]))
                    val_parts.append(np.concatenate(
                        [v, np.zeros(npad, np.float32)]))
        idx_cat = np.concatenate(idx_parts)
        colrel_cat = np.concatenate(colrel_parts)
        val_cat = np.concatenate(val_parts)
        if _os.environ.get("KERNEL_CLAMP_IDX"):
            idx_cat = (idx_cat % int(_os.environ["KERNEL_CLAMP_IDX"])).astype(np.int16)
        assert idx_cat.shape[0] == G_total * 128

        # wrapped idx stream, sliced per (sb, block, half) gather sub-run
        idx_w = []
        off = 0
        for s in range(NSB):
            for _h, ntok in sb_runs[s]:
                rem = ntok
                while rem > 0:
                    take = min(SUBRUN, rem)
                    idx_w.append(_wrap16(idx_cat[off:off + take]))
                    off += take
                    rem -= take
        idx_dram = (np.concatenate(idx_w, axis=1) if idx_w
                    else np.zeros((128, 16), np.int16))
        idx_cols = idx_dram.shape[1]

        colrel_dram = np.ascontiguousarray(
            colrel_cat.reshape(G_total, 128).T)
        val_dram = np.ascontiguousarray(val_cat.reshape(G_total, 128).T)

        xT = np.ascontiguousarray(xpad[c * NPC:c * NPC + NPCP].T)
        xroll = np.concatenate([xpad[c * NPC:N], xpad[:c * NPC], xpad[N:]])
        e_gate = np.ascontiguousarray(
            epad[c * NPC:c * NPC + NPCP].reshape(NB, 128, K)
            .transpose(1, 0, 2).reshape(128, NB * K))

        # every [128, X] constant packed into ONE int16 tensor: each bound
        # NEFF tensor costs ~28 us of dispatch overhead per execution, so
        # 7 inputs -> 1 is worth ~170 us/exec.  f32 regions first (4-byte
        # alignment), then bf16/i16; device side reads bitcast views.
        pack = np.concatenate([
            colrel_dram.view(np.int16),
            val_dram.view(np.int16),
            e_gate.view(np.int16),
            np.ascontiguousarray(xT.astype(BF16NP)).view(np.int16),
            np.ascontiguousarray(W_dram.astype(BF16NP)).view(np.int16),
            np.ascontiguousarray(
                np.tile(np.arange(128, dtype=np.float32),
                        (128, 1)).astype(BF16NP)).view(np.int16),
            np.ascontiguousarray(idx_dram),
        ], axis=1)

        in_maps.append({
            "x_full": np.ascontiguousarray(xroll),
            "pack": pack,
        })

    meta = dict(N=N, F=F, K=K, E=E, NPC=NPC, NB=NB, NPCP=NPCP, NSB=NSB,
                sb_nblocks=sb_nblocks, n_half=n_half, HALF=HALF,
                sched=sched, sb_runs=sb_runs, G_total=G_total,
                idx_cols=idx_cols,
                n_cores=n_cores)
    return meta, in_maps


def _build(meta):
    """Trace the Bass/Tile kernel for the static structure in meta."""
    from contextlib import ExitStack

    N, F, K = meta["N"], meta["F"], meta["K"]
    NB, NPCP, NSB = meta["NB"], meta["NPCP"], meta["NSB"]
    sb_nblocks, n_half, HALF = meta["sb_nblocks"], meta["n_half"], meta["HALF"]
    sched, sb_runs = meta["sched"], meta["sb_runs"]

    nc = bacc.Bacc("TRN2", target_bir_lowering=False, debug=False,
                   num_devices=meta["n_cores"], num_swdge_queues=4)

    x_full = nc.dram_tensor("x_full", [N + 128, F], F32, kind="ExternalInput")
    KF = K * F
    G = meta["G_total"]
    # packed constants, int16 columns (see _prep): colrel f32 | val f32 |
    # e_gate f32 | xT bf16 | W bf16 | iota bf16 | idx i16
    o_cr = 0
    o_val = o_cr + 2 * G
    o_eg = o_val + 2 * G
    o_xT = o_eg + 2 * NB * K
    o_w = o_xT + NPCP
    o_iota = o_w + 2 * KF
    o_idx = o_iota + 128
    PACK_C = o_idx + meta["idx_cols"]
    pack_d = nc.dram_tensor("pack", [128, PACK_C], I16, kind="ExternalInput")
    out_d = nc.dram_tensor("out", [NPCP, F], BF16, kind="ExternalOutput")

    assert KF == 1024 and F == 128, "einsum slicing hardcoded for K=8, F=128"

    with tile.TileContext(nc) as tc, ExitStack() as ctx:
        const = ctx.enter_context(tc.tile_pool(name="const", bufs=1))
        msgp = ctx.enter_context(tc.tile_pool(name="msg", bufs=14))
        msgbp = ctx.enter_context(tc.tile_pool(name="msgb", bufs=14))
        sp = ctx.enter_context(tc.tile_pool(name="sp", bufs=32))
        hiTp = ctx.enter_context(tc.tile_pool(name="hiT", bufs=NSB))
        accp = ctx.enter_context(tc.tile_pool(name="acc", bufs=6))
        psag = ctx.enter_context(tc.tile_pool(name="psag", bufs=3,
                                              space="PSUM"))
        psmm = ctx.enter_context(tc.tile_pool(name="psmm", bufs=4,
                                              space="PSUM"))

        # idx cols consumed by each superblock's gathers (for chunked loads)
        sb_idx_cols = []
        for s in range(NSB):
            cols = 0
            for _h, ntok in sb_runs[s]:
                rem = ntok
                while rem > 0:
                    take = min(SUBRUN, rem)
                    cols += take // 16
                    rem -= take
            sb_idx_cols.append(cols)

        # one persistent packed-constant tile; the idx region is loaded in
        # per-superblock chunks (first chunk first) so early gathers don't
        # wait on the full idx stream or the other constant loads.
        pack_t = const.tile([128, PACK_C], I16, tag="pack")
        off = 0
        for s in range(NSB):
            if sb_idx_cols[s]:
                nc.sync.dma_start(
                    pack_t[:, o_idx + off:o_idx + off + sb_idx_cols[s]],
                    pack_d.ap()[:, o_idx + off:o_idx + off + sb_idx_cols[s]])
                off += sb_idx_cols[s]
            if s == 0:
                nc.sync.dma_start(pack_t[:, 0:o_idx],
                                  pack_d.ap()[:, 0:o_idx])

        iota_v = pack_t[:, o_iota:o_iota + 128].bitcast(BF16)

        def cr_sc(g):
            return pack_t[:, o_cr + 2 * g:o_cr + 2 * g + 2].bitcast(F32)

        def val_sc(g):
            return pack_t[:, o_val + 2 * g:o_val + 2 * g + 2].bitcast(F32)

        def eg_sc(col):
            return pack_t[:, o_eg + 2 * col:o_eg + 2 * col + 2].bitcast(F32)

        def xT_view(a, b):
            return pack_t[:, o_xT + a:o_xT + b].bitcast(BF16)

        def w_view(a, b):
            return pack_t[:, o_w + a:o_w + b].bitcast(BF16)

        x_half = [x_full.ap()[h * HALF:N + 128, :] for h in range(n_half)]

        import os as _os
        _gather_only = _os.environ.get("KERNEL_GATHER_ONLY")
        _plain_dma = _os.environ.get("KERNEL_PLAIN_DMA")

        g_base = 0
        idx_off = 0
        n_gathers = 0
        for s in range(NSB):
            nb = sb_nblocks[s]
            groups = sched[s]
            ps_hi = psag.tile([128, nb * 128], F32, tag="psag")

            # gather msg tiles: one run per (block, half), in SUBRUN chunks
            tok_map = []            # token-order group idx -> (tile, slot)
            for h, ntok in sb_runs[s]:
                rem = ntok
                while rem > 0:
                    take = min(SUBRUN, rem)
                    mt = msgp.tile([128, take // 128, F], F32, tag="msg")
                    assert not _plain_dma, "plain-DMA probe retired"
                    nc.gpsimd.dma_gather(
                        mt[:], x_half[h],
                        pack_t[:, o_idx + idx_off:o_idx + idx_off + take // 16],
                        take, take, F, queue_num=n_gathers % 4)
                    n_gathers += 1
                    # cast to bf16 on the (otherwise idle) Act engine so the
                    # aggregation matmuls run at 1 cycle/row instead of 4
                    mb = msgbp.tile([128, take // 128, F], BF16, tag="msgb")
                    nc.scalar.copy(mb[:], mt[:])
                    for j in range(take // 128):
                        tok_map.append((mb, j))
                    idx_off += take // 16
                    rem -= take

            if _gather_only:
                for b_loc in range(nb):
                    b = s * SB_BLOCKS + b_loc
                    acc = accp.tile([128, F], BF16, tag="accb")
                    nc.vector.tensor_copy(acc[:], iota_v)
                    nc.sync.dma_start(out_d.ap()[b * 128:(b + 1) * 128, :],
                                      acc[:])
                g_base += len(groups)
                continue

            # S build + aggregation matmuls, block-major emission order
            for tg, b_loc, start, stop in groups:
                g = g_base + tg
                s_t = sp.tile([128, 128], BF16, tag="s")
                nc.vector.tensor_scalar(
                    s_t[:], iota_v, cr_sc(g), val_sc(g),
                    mybir.AluOpType.is_equal, mybir.AluOpType.mult)
                mt, j = tok_map[tg]
                nc.tensor.matmul(
                    ps_hi[:, b_loc * 128:(b_loc + 1) * 128],
                    mt[:, j:j + 1, :], s_t[:], start=start, stop=stop)
            g_base += len(groups)

            hiT_t = hiTp.tile([128, nb * 128], BF16, tag="hiT")
            nc.vector.tensor_copy(hiT_t[:], ps_hi[:])

            import os as _os
            if _os.environ.get("KERNEL_SKIP_EINSUM"):
                for b_loc in range(nb):
                    b = s * SB_BLOCKS + b_loc
                    acc = accp.tile([128, F], BF16, tag="accb")
                    nc.vector.tensor_copy(acc[:],
                                          hiT_t[:, b_loc * 128:(b_loc + 1) * 128])
                    nc.sync.dma_start(out_d.ap()[b * 128:(b + 1) * 128, :],
                                      acc[:])
                continue

            # einsum + gating + residual per block
            for b_loc in range(nb):
                b = s * SB_BLOCKS + b_loc
                hiT_b = hiT_t[:, b_loc * 128:(b_loc + 1) * 128]
                xT_b = xT_view(b * 128, (b + 1) * 128)
                pa = psmm.tile([128, 512], F32, tag="pmm")
                pb = psmm.tile([128, 512], F32, tag="pmm")
                nc.tensor.matmul(pa[:], hiT_b, w_view(0, 512),
                                 start=True, stop=False)
                nc.tensor.matmul(pb[:], hiT_b, w_view(512, 1024),
                                 start=True, stop=False)
                nc.tensor.matmul(pa[:], xT_b, w_view(1024, 1536),
                                 start=False, stop=True)
                nc.tensor.matmul(pb[:], xT_b, w_view(1536, 2048),
                                 start=False, stop=True)
                acc = accp.tile([128, F], F32, tag="acc")
                nc.sync.dma_start(acc[:], x_full.ap()[b * 128:(b + 1) * 128, :])
                acc_bf = accp.tile([128, F], BF16, tag="accb")
                for k in range(K):
                    src = pa if k < 4 else pb
                    kk = k % 4
                    dst = acc_bf if k == K - 1 else acc
                    nc.vector.scalar_tensor_tensor(
                        dst[:], src[:, kk * 128:(kk + 1) * 128],
                        eg_sc(b * K + k), acc[:],
                        mybir.AluOpType.mult, mybir.AluOpType.add)
                nc.sync.dma_start(out_d.ap()[b * 128:(b + 1) * 128, :],
                                  acc_bf[:])

    nc.compile()
    return nc


def _bench(nc, in_maps, n_cores, k_lo=2, k_hi=512, reps=3):
    """Amortized per-execution wall time of the compiled NEFF on the axon
    cores (inputs staged on device, pipelined async dispatches). Upper
    bound: includes axon per-dispatch overhead (~2.5-3 ms amortized).
    Returns (per_exec_ns, results_list)."""
    import time

    import jax
    from jax.sharding import Mesh, PartitionSpec
    from jax.experimental.shard_map import shard_map

    from concourse import bass2jax, mybir as _mb
    from concourse.bass2jax import _bass_exec_p, partition_id_tensor

    bass2jax.install_neuronx_cc_hook()

    partition_name = (nc.partition_id_tensor.name
                      if nc.partition_id_tensor else None)
    in_names, out_names, out_avals, zero_outs = [], [], [], []
    for alloc in nc.m.functions[0].allocations:
        if not isinstance(alloc, _mb.MemoryLocationSet):
            continue
        name = alloc.memorylocations[0].name
        if alloc.kind == "ExternalInput":
            if name != partition_name:
                in_names.append(name)
        elif alloc.kind == "ExternalOutput":
            shape = tuple(alloc.tensor_shape)
            dtype = _mb.dt.np(alloc.dtype)
            out_names.append(name)
            out_avals.append(jax.core.ShapedArray(shape, dtype))
            zero_outs.append(np.zeros(shape, dtype))
    n_params = len(in_names)
    all_in_names = in_names + out_names
    if partition_name is not None:
        all_in_names = all_in_names + [partition_name]
    def _body(*args):
        operands = list(args)
        if partition_name is not None:
            operands.append(partition_id_tensor())
        return tuple(_bass_exec_p.bind(
            *operands, out_avals=tuple(out_avals),
            in_names=tuple(all_in_names), out_names=tuple(out_names),
            lowering_input_output_aliases=(), sim_require_finite=True,
            sim_require_nnan=True, nc=nc))

    devices = jax.devices()[:n_cores]
    mesh = Mesh(np.asarray(devices), ("core",))
    nin = n_params + len(out_names)
    sh = jax.sharding.NamedSharding(mesh, PartitionSpec("core"))
    concat_in = [jax.device_put(
        np.concatenate([np.asarray(in_maps[c][k]) for c in range(n_cores)], 0),
        sh) for k in in_names]
    concat_zeros = [jax.device_put(
        np.zeros((n_cores * z.shape[0], *z.shape[1:]), z.dtype), sh)
        for z in zero_outs]
    fn = jax.jit(shard_map(_body, mesh=mesh,
                           in_specs=(PartitionSpec("core"),) * nin,
                           out_specs=(PartitionSpec("core"),) * len(out_names),
                           check_rep=False), keep_unused=True)
    out = fn(*concat_in, *concat_zeros)   # warmup (compile+load)
    jax.block_until_ready(out)
    iters = k_hi
    best = float("inf")
    for _ in range(reps):
        t0 = time.perf_counter()
        for _ in range(iters):
            out = fn(*concat_in, *concat_zeros)
        jax.block_until_ready(out)
        best = min(best, (time.perf_counter() - t0) / iters)
    results = [{name: np.asarray(out[i]).reshape(n_cores, *out_avals[i].shape)[c]
                for i, name in enumerate(out_names)} for c in range(n_cores)]
    return best * 1e9, results


def _run(x, adj, e, weights, n_cores=8, sim=False, trace=False):
    meta, in_maps = _prep(x, adj, e, weights, n_cores)
    nc = _build(meta)
    N, F, NPC, NPCP = meta["N"], meta["F"], meta["NPC"], meta["NPCP"]

    if sim:
        from concourse.bass_interp import CoreSim
        outs = []
        for c in range(n_cores):
            simr = CoreSim(nc)
            for k, v in in_maps[c].items():
                simr.tensor(k)[:] = v
            simr.simulate(check_with_hw=False)
            outs.append(np.array(simr.tensor("out")))
        res = None
    elif trace:
        per_iter_ns, results = _bench(nc, in_maps, n_cores)
        outs = [r["out"] for r in results]
        res = per_iter_ns
    else:
        from concourse.bass_utils import run_bass_kernel_spmd
        res = run_bass_kernel_spmd(nc, in_maps, core_ids=list(range(n_cores)),
                                   trace=trace)
        outs = [r["out"] for r in res.results]

    out = np.concatenate([o[:NPC] for o in outs], axis=0)
    assert out.shape == (N, F)
    return out.astype(np.float32), res


def kernel(x, adj, e, weights):
    x = np.asarray(x, dtype=np.float32)
    adj = np.asarray(adj)
    e = np.asarray(e, dtype=np.float32)
    weights = np.asarray(weights, dtype=np.float32)
    out, _ = _run(x, adj, e, weights, n_cores=8, sim=False)
    return out

